# revision 25
# baseline (speedup 1.0000x reference)
"""Causal multi-head self-attention (B=2, S=2048, D=768, H=12) on 8 TRN2 NeuronCores.

Sharding: core c = (batch b=c//4, head-group hg=c%4 of 3 heads).
Each core computes Q/K/V for its 3 heads, causal attention, and the partial
output projection sum_h out_h @ Wo[:, h]^T -> (S, D). Host sums the 4
head-group partials per batch (the unshard step).

HAM clock-gate: the PE's activity monitor only counts full-array matmuls
as "busy" — partial-array ones (64-row score stationaries, 65-col PV
stationaries) leave the clock gate at K=4/8 (1.2 GHz). So every matmul
is full 128x128-stationary:
  - scores: stationary = the full 128-row qkvt chunk (the cohabitant
    head's rows are neutralized by a zero-padded Q moving operand qz)
  - PV: stationary = vp = [V | ones | zeros] padded to 128 cols
  - out-proj: oct rows 64-127 of chunk 1 zeroed, stationary full-height
All matmul operands bf16 (fp16 streams at half rate); a 32-matmul warmup
chain on ident warms the clock gate while the first DMAs land.

Scoped overlap: scope 1 = [warmup, A(m0..m2), trV0, C(h0,qp0),
A(m3)+trV1, C(h0,qp1), A(m4)+trV2] with PSUM fill ring 2 banks + score
ring 2x2 + pout 2 — the trailing A chunks fill C(h0)'s exp-wait stalls
and keep the clock gate warm. Scope 2 = [C(h1), C(h2), D] with score
ring 3x2 + pout 2 (full attention needs the triple buffer to stay warm).
Output is written bf16 (host upcasts) to halve the output DMA.

C per head, per q-half qp (2 q-chunks of 512), per k-tile t:
  scoresT[k, q] = Kchunk.T @ qz   (only causally-valid halves)
  additive -30000 mask on the diagonal half, exp on ACT -> bf16,
  PV: pout[qc] += vp[t].T @ expT  (row 64 = denominator)
then per qc: recip(den) -> broadcast -> numerator * recip -> outcatT.
D: psum[q, j] += outcatT[:, q].T @ WoT[:, j]; ACT copy; DMA out.
"""

import numpy as np
import ml_dtypes
from contextlib import ExitStack

import concourse.bass as bass
import concourse.tile as tile
from concourse import bacc, mybir
from concourse import bass_utils

F32 = mybir.dt.float32
BF16 = mybir.dt.bfloat16
AF = mybir.ActivationFunctionType
BF = ml_dtypes.bfloat16

B, S, D, H = 2, 2048, 768, 12
DK = 64
HPC = 3            # heads per core
NCORES = 8
NI = D // 128      # 6 input-feature chunks
NM = 5             # output m-chunks of 128 (640 rows incl. 64 pad)
NT = S // 128      # 16 k-tiles
NQC = S // 512     # 4 q-chunks
MASK_NEG = -30000.0

# wcat m-chunks: m0=[q0;q1] m1=[k0;k1] m2=[q2;v0] m3=[k2;v1] m4=[v2;pad]
KCHUNK = [1, 1, 3]   # score stationary = full 128-row chunk holding K_h
VPOS = [(64, 2), (64, 3), (0, 4)]

_NC_CACHE = {}


def build_nc():
    if "nc" in _NC_CACHE:
        return _NC_CACHE["nc"]
    nc = bacc.Bacc("TRN2", target_bir_lowering=False, debug=False,
                   num_devices=NCORES)

    xt_d = nc.dram_tensor("xt", [NI, 128, S], BF16, kind="ExternalInput").ap()
    wcat_d = nc.dram_tensor("wcat", [NI, 128, NM * 128], BF16, kind="ExternalInput").ap()
    wot_d = nc.dram_tensor("wot", [2, 128, D], BF16, kind="ExternalInput").ap()
    mask_d = nc.dram_tensor("mask", [128, 128], F32, kind="ExternalInput").ap()
    id_d = nc.dram_tensor("ident", [128, 128], BF16, kind="ExternalInput").ap()
    out_d = nc.dram_tensor("out", [S, D], BF16, kind="ExternalOutput").ap()

    with tile.TileContext(nc) as tc, ExitStack() as ctx:
        const = ctx.enter_context(tc.tile_pool(name="const", bufs=1))

        # persistent SBUF buffers
        xt = const.tile([128, NI, S], BF16)             # X^T
        wcat = const.tile([128, NI, NM * 128], BF16)    # W^T (QKV packed)
        wot = const.tile([128, 2, D], BF16)             # Wo^T [h0;h1],[h2;0]
        maskb = const.tile([128, 128], F32)             # diag causal bias tile
        ident = const.tile([128, 128], BF16)
        qkvt = const.tile([128, NM, S], BF16)           # K^T/V^T chunks
        qz = const.tile([128, HPC, S], BF16)            # zero-padded Q^T per head
        vp = const.tile([128, HPC, NT, 128], BF16)      # V' = [V | ones | 0]
        oct_ = const.tile([128, 2, S], BF16)            # packed out^T [h0;h1],[h2;0]
        junk = const.tile([1, 16], F32)                 # keeps warmup chain live

        # DMA priority order = consumption order: weight chunk m lands just
        # before the xt shell that the first A-half of chunk m streams, so
        # the early A matmuls are never DMA-gated for long.
        nc.sync.dma_start(ident[:], id_d)
        for i in range(NI):
            nc.sync.dma_start(wcat[:, i, 0:128], wcat_d[i][:, 0:128])
        for i in range(NI):
            nc.sync.dma_start(xt[:, i, 0:512], xt_d[i][:, 0:512])
        for i in range(NI):
            nc.sync.dma_start(wcat[:, i, 128:256], wcat_d[i][:, 128:256])
        for i in range(NI):
            nc.sync.dma_start(xt[:, i, 512:1024], xt_d[i][:, 512:1024])
        for i in range(NI):
            nc.sync.dma_start(wcat[:, i, 256:384], wcat_d[i][:, 256:384])
        for sh in (2, 3):
            for i in range(NI):
                nc.sync.dma_start(xt[:, i, sh * 512:(sh + 1) * 512],
                                  xt_d[i][:, sh * 512:(sh + 1) * 512])
        for i in range(NI):
            nc.sync.dma_start(wcat[:, i, 384:NM * 128], wcat_d[i][:, 384:NM * 128])
        nc.sync.dma_start(maskb[:], mask_d)
        nc.sync.dma_start(wot[:], wot_d.rearrange("c p f -> p c f"))

        # zero/one fills on DVE, all disjoint from later writers (the vp pad
        # init deliberately avoids cols 0:DK so the V transposes never wait)
        nc.vector.memzero(qz[64:128, 0, :])
        nc.vector.memzero(qz[0:64, 1, :])
        nc.vector.memzero(qz[64:128, 2, :])
        nc.vector.memzero(oct_[64:128, 1, :])
        nc.vector.memzero(vp[:, :, :, DK:128])         # pad cols
        nc.vector.memset(vp[:, :, :, DK:DK + 1], 1.0)  # denominator ones col

        sb_exp = ctx.enter_context(tc.tile_pool(name="sb_exp", bufs=6))
        sb_div = ctx.enter_context(tc.tile_pool(name="sb_div", bufs=3))

        def a_half(ps, m, scp):
            pqs = [ps.tile([128, 512], F32, tag="fill",
                           name=f"pq{m}_{2 * scp + half}") for half in range(2)]
            for i in range(NI):
                for half in range(2):
                    sc = 2 * scp + half
                    nc.tensor.matmul(
                        pqs[half][:],
                        wcat[:, i, m * 128:(m + 1) * 128],
                        xt[:, i, sc * 512:(sc + 1) * 512],
                        start=(i == 0), stop=(i == NI - 1))
            for half in range(2):
                sc = 2 * scp + half
                s0, s1 = sc * 512, (sc + 1) * 512
                pq = pqs[half]
                if m == 0:      # pure Q chunk -> zero-padded q shadows
                    nc.vector.tensor_copy(qz[0:64, 0, s0:s1], pq[0:64, :])
                    nc.vector.tensor_copy(qz[64:128, 1, s0:s1], pq[64:128, :])
                elif m == 2:    # [q2; v0]
                    nc.vector.tensor_copy(qz[0:64, 2, s0:s1], pq[0:64, :])
                    nc.vector.tensor_copy(qkvt[64:128, m, s0:s1], pq[64:128, :])
                elif m == 4:    # [v2; pad]
                    nc.vector.tensor_copy(qkvt[0:64, m, s0:s1], pq[0:64, :])
                else:           # full K chunks (score stationaries)
                    nc.vector.tensor_copy(qkvt[:, m, s0:s1], pq[:])

        kw_n = [0]

        def keepalive(ps, tag, width, n=6):
            # always-ready full-array matmul chain; the scheduler slots it
            # into predicted PE stalls, keeping the HAM clock-gate warm
            # across block transitions without delaying ready real work much
            kw = ps.tile([128, width], F32, tag=tag, name=f"kw{kw_n[0]}")
            kw_n[0] += 1
            for j in range(n):
                nc.tensor.matmul(kw[:, 0:128], ident[:], ident[:],
                                 start=(j == 0), stop=(j == n - 1))
            nc.vector.tensor_copy(junk[:], kw[0:1, 0:16])

        def tr_block(ps, h):
            vb, vchunk = VPOS[h]
            for t in range(NT):
                ptr = ps.tile([128, DK], BF16, tag="fill", name=f"tr{h}_{t}")
                nc.tensor.transpose(
                    ptr[:], qkvt[vb:vb + DK, vchunk, t * 128:(t + 1) * 128],
                    ident[vb:vb + DK, vb:vb + DK])
                nc.vector.tensor_copy(vp[:, h, t, 0:DK], ptr[:])

        def c_block(ps_s, ps_o, h, qp):
            kchunk = KCHUNK[h]
            pouts = {}

            def score_step(t):
                qcs = (2 * qp, 2 * qp + 1)
                qc_lo = t // 4
                off = 128 * (t % 4)   # diag col offset inside qc_lo's half
                pscr = ps_s.tile([128, 1024], F32, tag="scr",
                                 name=f"sc{h}_{qp}_{t}")
                for half, qc in enumerate(qcs):
                    if qc < qc_lo:
                        continue
                    cs = off if qc == qc_lo else 0  # skip fully-masked cols
                    nc.tensor.matmul(
                        pscr[:, half * 512 + cs:(half + 1) * 512],
                        qkvt[:, kchunk, t * 128:(t + 1) * 128],
                        qz[:, h, qc * 512 + cs:(qc + 1) * 512],
                        start=True, stop=True)
                if qc_lo in qcs:  # mask only the 128-wide diagonal window
                    half = qc_lo - 2 * qp
                    nc.vector.tensor_add(
                        pscr[:, half * 512 + off:half * 512 + off + 128],
                        pscr[:, half * 512 + off:half * 512 + off + 128],
                        maskb[:, 0:128])
                lo = (512 if qc_lo == qcs[1] else 0) + \
                     (off if qc_lo in qcs else 0)
                expt = sb_exp.tile([128, 1024], BF16, tag="exp",
                                   name=f"ex{h}_{qp}_{t}")
                nc.scalar.activation(expt[:, lo:1024], pscr[:, lo:1024],
                                     AF.Exp)
                return expt

            def pv_step(t, expt):
                qcs = (2 * qp, 2 * qp + 1)
                qc_lo = t // 4
                off = 128 * (t % 4)
                for half, qc in enumerate(qcs):
                    if qc < qc_lo:
                        continue
                    cs = off if qc == qc_lo else 0
                    nc.tensor.matmul(
                        pouts[qc][:, cs:512],
                        vp[:, h, t, :],
                        expt[:, half * 512 + cs:(half + 1) * 512],
                        start=(t == 0), stop=(t == 4 * qc + 3))

            def divide(qc):
                # evict the finished chain so its PSUM bank frees; the slow
                # recip/divide runs off the copy
                nout = sb_div.tile([DK + 1, 512], F32, tag="nout",
                                   name=f"no{h}_{qc}")
                nc.vector.tensor_copy(nout[:], pouts[qc][0:DK + 1, :])
                # spread the 512-wide den row over 64 partitions so the
                # expensive reciprocal runs 64 lanes wide, not 1
                rsp = sb_div.tile([DK, 8], F32, tag="rsp", name=f"rsp{h}_{qc}")
                nc.sync.dma_start(rsp[:], nout[DK:DK + 1, :])
                rcs = sb_div.tile([DK, 8], F32, tag="rcs", name=f"rcs{h}_{qc}")
                nc.vector.reciprocal(rcs[:], rsp[:])
                rc0 = sb_div.tile([1, 512], F32, tag="rc0", name=f"rc0{h}_{qc}")
                nc.sync.dma_start(rc0[:], rcs[:])
                rb = sb_div.tile([DK, 512], F32, tag="rb", name=f"rb{h}_{qc}")
                nc.gpsimd.partition_broadcast(rb[:], rc0[:])
                if h == 1:
                    # h1 lands at partitions 64-127: shift via SBUF DMA
                    tmp = sb_div.tile([DK, 512], BF16, tag="tmp",
                                      name=f"tmp{h}_{qc}")
                    nc.vector.tensor_mul(tmp[:], nout[0:DK, :], rb[:])
                    nc.sync.dma_start(
                        oct_[DK:128, 0, qc * 512:(qc + 1) * 512], tmp[:])
                else:
                    nc.vector.tensor_mul(
                        oct_[0:DK, h // 2, qc * 512:(qc + 1) * 512],
                        nout[0:DK, :], rb[:])

            for qc in (2 * qp, 2 * qp + 1):
                pouts[qc] = ps_o.tile([128, 512], F32, tag="pout",
                                      name=f"po{h}_{qc}")
            for t0 in range(0, 4 * (2 * qp + 1) + 4, 2):
                e0 = score_step(t0)
                e1 = score_step(t0 + 1)
                pv_step(t0, e0)
                pv_step(t0 + 1, e1)
                if t0 + 1 == 4 * (2 * qp) + 3:
                    divide(2 * qp)      # low chain done: free its bank
            divide(2 * qp + 1)

        def d_block(ps_s, qts, act_copy=True):
            for qt in qts:
                pp = ps_s.tile([128, D], F32, tag="scr", name=f"pp{qt}")
                for c in (0, 1):
                    for js, je in ((0, 512), (512, D)):
                        nc.tensor.matmul(
                            pp[:, js:je],
                            oct_[:, c, qt * 128:(qt + 1) * 128],
                            wot[:, c, js:je],
                            start=(c == 0), stop=(c == 1))
                ot = sb_exp.tile([128, D], BF16, tag="exp", name=f"ot{qt}")
                if act_copy:
                    nc.scalar.copy(ot[:], pp[:])    # ACT is idle during D
                else:
                    # interleaved with C: ACT is busy with exps, use DVE
                    nc.vector.tensor_copy(ot[:], pp[:])
                nc.sync.dma_start(out_d[qt * 128:(qt + 1) * 128, :], ot[:])

        # ---- scope 1: warmup + A + C(h0); trailing A chunks fill C(h0)'s
        # exp-wait stalls and keep the clock-gate warm across the boundary
        with tc.tile_pool(name="ps_f", bufs=2, space="PSUM") as ps_f, \
             tc.tile_pool(name="ps_s1", bufs=2, space="PSUM") as ps_s1, \
             tc.tile_pool(name="ps_o1", bufs=2, space="PSUM") as ps_o1:
            # warmup: covers the HAM SHORT window (~3.4us) plus the first
            # DMA latency, so A's first chains run at 2.4 GHz
            pw = ps_f.tile([128, 512], F32, tag="fill", name="warm")
            for j in range(48):
                nc.tensor.matmul(pw[:, 0:128], ident[:], ident[:],
                                 start=(j == 0), stop=(j == 47))
            nc.vector.tensor_copy(junk[:], pw[0:1, 0:16])  # defeat DCE

            # first halves only need the first two xt shells; staggered with
            # the weight-chunk DMAs this keeps the PE stream gap-free
            a_half(ps_f, 0, 0)
            a_half(ps_f, 1, 0)
            a_half(ps_f, 2, 0)
            a_half(ps_f, 0, 1)
            a_half(ps_f, 1, 1)
            a_half(ps_f, 2, 1)
            tr_block(ps_f, 0)
            c_block(ps_s1, ps_o1, 0, 0)
            a_half(ps_f, 3, 0)
            a_half(ps_f, 3, 1)
            tr_block(ps_f, 1)
            c_block(ps_s1, ps_o1, 0, 1)
            a_half(ps_f, 4, 0)
            a_half(ps_f, 4, 1)
            tr_block(ps_f, 2)
            keepalive(ps_f, "fill", 512, n=8)

        # ---- scope 2: remaining heads (triple-buffered scores) + out-proj
        with tc.tile_pool(name="ps_s2", bufs=3, space="PSUM") as ps_s2, \
             tc.tile_pool(name="ps_o2", bufs=2, space="PSUM") as ps_o2:
            c_block(ps_s2, ps_o2, 1, 0)
            keepalive(ps_s2, "scr", 1024, n=8)
            c_block(ps_s2, ps_o2, 1, 1)
            keepalive(ps_s2, "scr", 1024, n=6)
            c_block(ps_s2, ps_o2, 2, 0)
            keepalive(ps_s2, "scr", 1024, n=6)
            c_block(ps_s2, ps_o2, 2, 1)
            keepalive(ps_s2, "scr", 1024, n=8)
            d_block(ps_s2, range(NT))

    nc.compile()
    _NC_CACHE["nc"] = nc
    return nc


def make_in_maps(X, Wq, Wk, Wv, Wo):
    X = np.ascontiguousarray(np.asarray(X, dtype=np.float32))
    Wq = np.asarray(Wq, dtype=np.float32)
    Wk = np.asarray(Wk, dtype=np.float32)
    Wv = np.asarray(Wv, dtype=np.float32)
    Wo = np.asarray(Wo, dtype=np.float32)

    # causal additive-bias tiles: keep q >= k; rows=k (p), cols=q (f)
    p = np.arange(128)[:, None]
    f = np.arange(512)[None, :]
    mask = np.where(f[:, :128] >= p, 0.0, MASK_NEG).astype(np.float32)
    ident = np.eye(128, dtype=np.float32).astype(BF)

    in_maps = []
    for c in range(NCORES):
        b, hg = c // 4, c % 4
        gh = [hg * HPC + l for l in range(HPC)]
        q = [Wq[g * DK:(g + 1) * DK, :] / 8.0 for g in gh]
        k = [Wk[g * DK:(g + 1) * DK, :] for g in gh]
        v = [Wv[g * DK:(g + 1) * DK, :] for g in gh]
        wcat_rows = np.vstack([
            q[0], q[1], k[0], k[1], q[2], v[0], k[2], v[1], v[2],
            np.zeros((DK, D), dtype=np.float32),
        ])                                            # (640, 768)
        wcat = np.ascontiguousarray(
            wcat_rows.T.reshape(NI, 128, NM * 128)).astype(BF)
        w0, w1, w2 = (Wo[:, g * DK:(g + 1) * DK].T for g in gh)
        wot = np.ascontiguousarray(np.stack([
            np.vstack([w0, w1]),
            np.vstack([w2, np.zeros((DK, D), dtype=np.float32)]),
        ])).astype(BF)                                # (2, 128, 768)
        xt = np.ascontiguousarray(X[b].T.reshape(NI, 128, S)).astype(BF)
        in_maps.append({
            "xt": xt, "wcat": wcat, "wot": wot,
            "mask": mask, "ident": ident,
        })
    return in_maps


def _run(in_maps, trace=False, trace_cores=None):
    nc = build_nc()
    return bass_utils.run_bass_kernel_spmd(
        nc, in_maps, core_ids=list(range(NCORES)),
        trace=trace, trace_cores=trace_cores,
    )


def kernel(X, Wq, Wk, Wv, Wo):
    in_maps = make_in_maps(X, Wq, Wk, Wv, Wo)
    res = _run(in_maps, trace=False)
    out = np.zeros((B, S, D), dtype=np.float32)
    for c in range(NCORES):
        out[c // 4] += np.asarray(res.results[c]["out"], dtype=np.float32)
    return out


# revision 30
# speedup vs baseline: 1.0329x; 1.0329x over previous
"""Causal multi-head self-attention (B=2, S=2048, D=768, H=12) on 8 TRN2 NeuronCores.

Sharding: core c = (batch b=c//4, head-group hg=c%4 of 3 heads).
Each core computes Q/K/V for its 3 heads, causal attention, and the partial
output projection sum_h out_h @ Wo[:, h]^T -> (S, D). Host sums the 4
head-group partials per batch (the unshard step).

HAM clock-gate: the PE's activity monitor only counts full-array matmuls
as "busy" — partial-array ones (64-row score stationaries, 65-col PV
stationaries) leave the clock gate at K=4/8 (1.2 GHz). So every matmul
is full 128x128-stationary:
  - scores: stationary = the full 128-row qkvt chunk (the cohabitant
    head's rows are neutralized by a zero-padded Q moving operand qz)
  - PV: stationary = vp = [V | ones | zeros] padded to 128 cols
  - out-proj: oct rows 64-127 of chunk 1 zeroed, stationary full-height
All matmul operands bf16 (fp16 streams at half rate); a 32-matmul warmup
chain on ident warms the clock gate while the first DMAs land.

Scoped overlap: scope 1 = [warmup, A(m0..m2), trV0, C(h0,qp0),
A(m3)+trV1, C(h0,qp1), A(m4)+trV2] with PSUM fill ring 2 banks + score
ring 2x2 + pout 2 — the trailing A chunks fill C(h0)'s exp-wait stalls
and keep the clock gate warm. Scope 2 = [C(h1), C(h2), D] with score
ring 3x2 + pout 2 (full attention needs the triple buffer to stay warm).
Output is written bf16 (host upcasts) to halve the output DMA.

C per head, per q-half qp (2 q-chunks of 512), per k-tile t:
  scoresT[k, q] = Kchunk.T @ qz   (only causally-valid halves)
  additive -30000 mask on the diagonal half, exp on ACT -> bf16,
  PV: pout[qc] += vp[t].T @ expT  (row 64 = denominator)
then per qc: recip(den) -> broadcast -> numerator * recip -> outcatT.
D: psum[q, j] += outcatT[:, q].T @ WoT[:, j]; ACT copy; DMA out.
"""

import numpy as np
import ml_dtypes
from contextlib import ExitStack

import concourse.bass as bass
import concourse.tile as tile
from concourse import bacc, mybir
from concourse import bass_utils

F32 = mybir.dt.float32
BF16 = mybir.dt.bfloat16
AF = mybir.ActivationFunctionType
BF = ml_dtypes.bfloat16

B, S, D, H = 2, 2048, 768, 12
DK = 64
HPC = 3            # heads per core
NCORES = 8
NI = D // 128      # 6 input-feature chunks
NM = 5             # output m-chunks of 128 (640 rows incl. 64 pad)
NT = S // 128      # 16 k-tiles
NQC = S // 512     # 4 q-chunks
MASK_NEG = -30000.0

# wcat m-chunks: m0=[q0;q1] m1=[k0;k1] m2=[q2;v0] m3=[k2;v1] m4=[v2;pad]
KCHUNK = [1, 1, 3]   # score stationary = full 128-row chunk holding K_h
VPOS = [(64, 2), (64, 3), (0, 4)]

_NC_CACHE = {}


def build_nc():
    if "nc" in _NC_CACHE:
        return _NC_CACHE["nc"]
    nc = bacc.Bacc("TRN2", target_bir_lowering=False, debug=False,
                   num_devices=NCORES)

    xt_d = nc.dram_tensor("xt", [NI, 128, S], BF16, kind="ExternalInput").ap()
    wcat_d = nc.dram_tensor("wcat", [NI, 128, NM * 128], BF16, kind="ExternalInput").ap()
    wot_d = nc.dram_tensor("wot", [2, 128, D], BF16, kind="ExternalInput").ap()
    mask_d = nc.dram_tensor("mask", [128, 128], F32, kind="ExternalInput").ap()
    id_d = nc.dram_tensor("ident", [128, 128], BF16, kind="ExternalInput").ap()
    out_d = nc.dram_tensor("out", [S, D], BF16, kind="ExternalOutput").ap()

    with tile.TileContext(nc) as tc, ExitStack() as ctx:
        const = ctx.enter_context(tc.tile_pool(name="const", bufs=1))

        # persistent SBUF buffers
        xt = const.tile([128, NI, S], BF16)             # X^T
        wcat = const.tile([128, NI, NM * 128], BF16)    # W^T (QKV packed)
        wot = const.tile([128, 2, D], BF16)             # Wo^T [h0;h1],[h2;0]
        maskb = const.tile([128, 128], F32)             # diag causal bias tile
        ident = const.tile([128, 128], BF16)
        qkvt = const.tile([128, NM, S], BF16)           # K^T/V^T chunks
        qz = const.tile([128, HPC, S], BF16)            # zero-padded Q^T per head
        vp = const.tile([128, HPC, NT, 128], BF16)      # V' = [V | ones | 0]
        oct_ = const.tile([128, 2, S], BF16)            # packed out^T [h0;h1],[h2;0]
        junk = const.tile([1, 16], F32)                 # keeps warmup chain live
        wj = const.tile([128, 128], BF16)               # warmup stationary

        # DMA priority order = consumption order: weight chunk m lands just
        # before the xt shell that the first A-half of chunk m streams, so
        # the early A matmuls are never DMA-gated for long. ident is only
        # needed by the transposes (~20us in), so it loads late.
        for i in range(NI):
            nc.sync.dma_start(wcat[:, i, 0:128], wcat_d[i][:, 0:128])
        for i in range(NI):
            nc.sync.dma_start(xt[:, i, 0:512], xt_d[i][:, 0:512])
        for i in range(NI):
            nc.sync.dma_start(wcat[:, i, 128:256], wcat_d[i][:, 128:256])
        for i in range(NI):
            nc.sync.dma_start(xt[:, i, 512:1024], xt_d[i][:, 512:1024])
        for i in range(NI):
            nc.sync.dma_start(wcat[:, i, 256:384], wcat_d[i][:, 256:384])
        for sh in (2, 3):
            for i in range(NI):
                nc.sync.dma_start(xt[:, i, sh * 512:(sh + 1) * 512],
                                  xt_d[i][:, sh * 512:(sh + 1) * 512])
        for i in range(NI):
            nc.sync.dma_start(wcat[:, i, 384:NM * 128], wcat_d[i][:, 384:NM * 128])
        nc.sync.dma_start(ident[:], id_d)
        nc.sync.dma_start(maskb[:], mask_d)
        nc.sync.dma_start(wot[:], wot_d.rearrange("c p f -> p c f"))

        # warmup stationary: DVE-memset (no DMA dependency) so the warmup
        # chain starts ~immediately instead of waiting for the DMA queues
        nc.vector.memset(wj[:], 0.25)
        # zero/one fills on DVE, all disjoint from later writers (the vp pad
        # init deliberately avoids cols 0:DK so the V transposes never wait)
        nc.vector.memzero(qz[64:128, 0, :])
        nc.vector.memzero(qz[0:64, 1, :])
        nc.vector.memzero(qz[64:128, 2, :])
        nc.vector.memzero(oct_[64:128, 1, :])
        nc.vector.memzero(vp[:, :, :, DK:128])         # pad cols
        nc.vector.memset(vp[:, :, :, DK:DK + 1], 1.0)  # denominator ones col

        sb_exp = ctx.enter_context(tc.tile_pool(name="sb_exp", bufs=6))
        sb_div = ctx.enter_context(tc.tile_pool(name="sb_div", bufs=3))

        def a_half(ps, m, scp):
            pqs = [ps.tile([128, 512], F32, tag="fill",
                           name=f"pq{m}_{2 * scp + half}") for half in range(2)]
            for i in range(NI):
                for half in range(2):
                    sc = 2 * scp + half
                    nc.tensor.matmul(
                        pqs[half][:],
                        wcat[:, i, m * 128:(m + 1) * 128],
                        xt[:, i, sc * 512:(sc + 1) * 512],
                        start=(i == 0), stop=(i == NI - 1))
            for half in range(2):
                sc = 2 * scp + half
                s0, s1 = sc * 512, (sc + 1) * 512
                pq = pqs[half]
                if m == 0:      # pure Q chunk -> zero-padded q shadows
                    nc.vector.tensor_copy(qz[0:64, 0, s0:s1], pq[0:64, :])
                    nc.vector.tensor_copy(qz[64:128, 1, s0:s1], pq[64:128, :])
                elif m == 2:    # [q2; v0]
                    nc.vector.tensor_copy(qz[0:64, 2, s0:s1], pq[0:64, :])
                    nc.vector.tensor_copy(qkvt[64:128, m, s0:s1], pq[64:128, :])
                elif m == 4:    # [v2; pad]
                    nc.vector.tensor_copy(qkvt[0:64, m, s0:s1], pq[0:64, :])
                else:           # full K chunks (score stationaries)
                    nc.vector.tensor_copy(qkvt[:, m, s0:s1], pq[:])

        def tr_block(ps, h):
            vb, vchunk = VPOS[h]
            for t in range(NT):
                ptr = ps.tile([128, DK], BF16, tag="fill", name=f"tr{h}_{t}")
                nc.tensor.transpose(
                    ptr[:], qkvt[vb:vb + DK, vchunk, t * 128:(t + 1) * 128],
                    ident[vb:vb + DK, vb:vb + DK])
                nc.vector.tensor_copy(vp[:, h, t, 0:DK], ptr[:])

        def c_block(ps_s, ps_o, h, qp):
            kchunk = KCHUNK[h]
            pouts = {}

            def score_step(t):
                qcs = (2 * qp, 2 * qp + 1)
                qc_lo = t // 4
                off = 128 * (t % 4)   # diag col offset inside qc_lo's half
                pscr = ps_s.tile([128, 1024], F32, tag="scr",
                                 name=f"sc{h}_{qp}_{t}")
                for half, qc in enumerate(qcs):
                    if qc < qc_lo:
                        continue
                    cs = off if qc == qc_lo else 0  # skip fully-masked cols
                    nc.tensor.matmul(
                        pscr[:, half * 512 + cs:(half + 1) * 512],
                        qkvt[:, kchunk, t * 128:(t + 1) * 128],
                        qz[:, h, qc * 512 + cs:(qc + 1) * 512],
                        start=True, stop=True)
                if qc_lo in qcs:  # mask only the 128-wide diagonal window
                    half = qc_lo - 2 * qp
                    nc.vector.tensor_add(
                        pscr[:, half * 512 + off:half * 512 + off + 128],
                        pscr[:, half * 512 + off:half * 512 + off + 128],
                        maskb[:, 0:128])
                lo = (512 if qc_lo == qcs[1] else 0) + \
                     (off if qc_lo in qcs else 0)
                expt = sb_exp.tile([128, 1024], BF16, tag="exp",
                                   name=f"ex{h}_{qp}_{t}")
                nc.scalar.activation(expt[:, lo:1024], pscr[:, lo:1024],
                                     AF.Exp)
                return expt

            def pv_step(t, expt):
                qcs = (2 * qp, 2 * qp + 1)
                qc_lo = t // 4
                off = 128 * (t % 4)
                for half, qc in enumerate(qcs):
                    if qc < qc_lo:
                        continue
                    cs = off if qc == qc_lo else 0
                    nc.tensor.matmul(
                        pouts[qc][:, cs:512],
                        vp[:, h, t, :],
                        expt[:, half * 512 + cs:(half + 1) * 512],
                        start=(t == 0), stop=(t == 4 * qc + 3))

            def divide(qc):
                # evict the finished chain so its PSUM bank frees; the slow
                # recip/divide runs off the copy
                nout = sb_div.tile([DK + 1, 512], F32, tag="nout",
                                   name=f"no{h}_{qc}")
                nc.vector.tensor_copy(nout[:], pouts[qc][0:DK + 1, :])
                # spread the 512-wide den row over 64 partitions so the
                # expensive reciprocal runs 64 lanes wide, not 1
                rsp = sb_div.tile([DK, 8], F32, tag="rsp", name=f"rsp{h}_{qc}")
                nc.sync.dma_start(rsp[:], nout[DK:DK + 1, :])
                rcs = sb_div.tile([DK, 8], F32, tag="rcs", name=f"rcs{h}_{qc}")
                nc.vector.reciprocal(rcs[:], rsp[:])
                rc0 = sb_div.tile([1, 512], F32, tag="rc0", name=f"rc0{h}_{qc}")
                nc.sync.dma_start(rc0[:], rcs[:])
                rb = sb_div.tile([DK, 512], F32, tag="rb", name=f"rb{h}_{qc}")
                nc.gpsimd.partition_broadcast(rb[:], rc0[:])
                if h == 1:
                    # h1 lands at partitions 64-127: shift via SBUF DMA
                    tmp = sb_div.tile([DK, 512], BF16, tag="tmp",
                                      name=f"tmp{h}_{qc}")
                    nc.vector.tensor_mul(tmp[:], nout[0:DK, :], rb[:])
                    nc.sync.dma_start(
                        oct_[DK:128, 0, qc * 512:(qc + 1) * 512], tmp[:])
                else:
                    nc.vector.tensor_mul(
                        oct_[0:DK, h // 2, qc * 512:(qc + 1) * 512],
                        nout[0:DK, :], rb[:])

            for qc in (2 * qp, 2 * qp + 1):
                pouts[qc] = ps_o.tile([128, 512], F32, tag="pout",
                                      name=f"po{h}_{qc}")
            for t0 in range(0, 4 * (2 * qp + 1) + 4, 2):
                e0 = score_step(t0)
                e1 = score_step(t0 + 1)
                pv_step(t0, e0)
                pv_step(t0 + 1, e1)
                if t0 + 1 == 4 * (2 * qp) + 3:
                    divide(2 * qp)      # low chain done: free its bank
            divide(2 * qp + 1)

        def d_block(ps_s, qts, act_copy=True):
            for qt in qts:
                pp = ps_s.tile([128, D], F32, tag="scr", name=f"pp{qt}")
                for c in (0, 1):
                    for js, je in ((0, 512), (512, D)):
                        nc.tensor.matmul(
                            pp[:, js:je],
                            oct_[:, c, qt * 128:(qt + 1) * 128],
                            wot[:, c, js:je],
                            start=(c == 0), stop=(c == 1))
                ot = sb_exp.tile([128, D], BF16, tag="exp", name=f"ot{qt}")
                if act_copy:
                    nc.scalar.copy(ot[:], pp[:])    # ACT is idle during D
                else:
                    # interleaved with C: ACT is busy with exps, use DVE
                    nc.vector.tensor_copy(ot[:], pp[:])
                nc.sync.dma_start(out_d[qt * 128:(qt + 1) * 128, :], ot[:])

        # ---- scope 1: warmup + A + C(h0); trailing A chunks fill C(h0)'s
        # exp-wait stalls and keep the clock-gate warm across the boundary
        with tc.tile_pool(name="ps_f", bufs=2, space="PSUM") as ps_f, \
             tc.tile_pool(name="ps_s1", bufs=2, space="PSUM") as ps_s1, \
             tc.tile_pool(name="ps_o1", bufs=2, space="PSUM") as ps_o1:
            # warmup: spans the whole DMA-gated head (~9us) so the PE is warm
            # and busy from ~0.3us until A's first operands land
            pw = ps_f.tile([128, 512], F32, tag="fill", name="warm")
            for j in range(120):
                nc.tensor.matmul(pw[:, 0:128], wj[:], wj[:],
                                 start=(j == 0), stop=(j == 119))
            nc.vector.tensor_copy(junk[:], pw[0:1, 0:16])  # defeat DCE

            # first halves only need the first two xt shells; staggered with
            # the weight-chunk DMAs this keeps the PE stream gap-free
            a_half(ps_f, 0, 0)
            a_half(ps_f, 1, 0)
            a_half(ps_f, 2, 0)
            a_half(ps_f, 0, 1)
            a_half(ps_f, 1, 1)
            a_half(ps_f, 2, 1)
            tr_block(ps_f, 0)
            c_block(ps_s1, ps_o1, 0, 0)
            a_half(ps_f, 3, 0)
            a_half(ps_f, 3, 1)
            tr_block(ps_f, 1)
            c_block(ps_s1, ps_o1, 0, 1)
            a_half(ps_f, 4, 0)
            a_half(ps_f, 4, 1)
            tr_block(ps_f, 2)

        # ---- scope 2: remaining heads (triple-buffered scores) + out-proj
        with tc.tile_pool(name="ps_s2", bufs=3, space="PSUM") as ps_s2, \
             tc.tile_pool(name="ps_o2", bufs=2, space="PSUM") as ps_o2:
            c_block(ps_s2, ps_o2, 1, 0)
            c_block(ps_s2, ps_o2, 1, 1)
            c_block(ps_s2, ps_o2, 2, 0)
            c_block(ps_s2, ps_o2, 2, 1)
            d_block(ps_s2, range(NT))

    nc.compile()
    _NC_CACHE["nc"] = nc
    return nc


def make_in_maps(X, Wq, Wk, Wv, Wo):
    X = np.ascontiguousarray(np.asarray(X, dtype=np.float32))
    Wq = np.asarray(Wq, dtype=np.float32)
    Wk = np.asarray(Wk, dtype=np.float32)
    Wv = np.asarray(Wv, dtype=np.float32)
    Wo = np.asarray(Wo, dtype=np.float32)

    # causal additive-bias tiles: keep q >= k; rows=k (p), cols=q (f)
    p = np.arange(128)[:, None]
    f = np.arange(512)[None, :]
    mask = np.where(f[:, :128] >= p, 0.0, MASK_NEG).astype(np.float32)
    ident = np.eye(128, dtype=np.float32).astype(BF)

    in_maps = []
    for c in range(NCORES):
        b, hg = c // 4, c % 4
        gh = [hg * HPC + l for l in range(HPC)]
        q = [Wq[g * DK:(g + 1) * DK, :] / 8.0 for g in gh]
        k = [Wk[g * DK:(g + 1) * DK, :] for g in gh]
        v = [Wv[g * DK:(g + 1) * DK, :] for g in gh]
        wcat_rows = np.vstack([
            q[0], q[1], k[0], k[1], q[2], v[0], k[2], v[1], v[2],
            np.zeros((DK, D), dtype=np.float32),
        ])                                            # (640, 768)
        wcat = np.ascontiguousarray(
            wcat_rows.T.reshape(NI, 128, NM * 128)).astype(BF)
        w0, w1, w2 = (Wo[:, g * DK:(g + 1) * DK].T for g in gh)
        wot = np.ascontiguousarray(np.stack([
            np.vstack([w0, w1]),
            np.vstack([w2, np.zeros((DK, D), dtype=np.float32)]),
        ])).astype(BF)                                # (2, 128, 768)
        xt = np.ascontiguousarray(X[b].T.reshape(NI, 128, S)).astype(BF)
        in_maps.append({
            "xt": xt, "wcat": wcat, "wot": wot,
            "mask": mask, "ident": ident,
        })
    return in_maps


def _run(in_maps, trace=False, trace_cores=None):
    nc = build_nc()
    return bass_utils.run_bass_kernel_spmd(
        nc, in_maps, core_ids=list(range(NCORES)),
        trace=trace, trace_cores=trace_cores,
    )


def kernel(X, Wq, Wk, Wv, Wo):
    in_maps = make_in_maps(X, Wq, Wk, Wv, Wo)
    res = _run(in_maps, trace=False)
    out = np.zeros((B, S, D), dtype=np.float32)
    for c in range(NCORES):
        out[c // 4] += np.asarray(res.results[c]["out"], dtype=np.float32)
    return out
